# revision 4
# baseline (speedup 1.0000x reference)
"""Trainium2 Bass kernel for a full MHA transformer block (B=8, data-parallel).

Per core (one batch element):
    qh/kh/vh = x @ W  (+zero bias), 16 heads x 64
    attn     = softmax(qh @ kh^T / 8)
    out      = LayerNorm(gelu(ctx @ Wo) + residual)

v2 design (scheduling rewrite vs the PE-transpose baseline):
  - Inputs arrive HOST-pre-transposed as X^T [d, s] bf16 -> no PE transposes.
  - DMA priority: K-side, V-side, Q-side, wo (wo reuses wk's SBUF slot, so
    its DMA is WAR-gated until Kproj finishes).
  - PE: Kproj*8 -> Qproj(0) -> Vproj*8 (scores for unit 0 woven) -> a
    16-unit steady loop (unit = (q-half, head-pair)): each iteration emits
    the unit's 8 scores tiles singly spaced between Qproj halves / ctx
    pairs / outproj quarters so the single scores-PSUM buffer round-trips
    through exp without stalling PE.
  - exp split: ScalarE activation for most kt, VectorE Schraudolph
    (i16 = A*s + B, bitcast to bf16) for the last kt(s) of each unit.
  - softmax denominator via the ones column appended to V; normalize is
    software-pipelined: [row-copy, partition-move DMA, broadcast] one unit
    ahead of [reciprocal, muls] so the cross-partition latency overlaps.
  - out-proj both-halves PSUM first, then both pre-gelu copies parked into
    the dead ct[:, :, ssl] bytes; ALL gelu deferred after the last exp
    (ACT executes in order -> exactly two activation-table switches).
  - y2 (gelu+resid, LN) aliased onto the dead qt tile; output stored bf16,
    host upcasts to f32.
"""

import numpy as np

S, D, H, DH = 1024, 1024, 16, 64
EPS = 1e-5
NCORES = 8
P = 128
SC = S // P    # seq chunks (8)
DC = D // P    # feature chunks (8)
HP = H // 2    # head pairs (8)

# Schraudolph exp as bf16 bits: i16 = A*s + B (s = raw score; the 1/8
# softmax scale is folded into A), bitcast int16 -> bf16.
EXP_A = 23.083120654223414
EXP_B = 16250.585

# which kt tiles go to the DVE instead of ACT, per q-half
DVE_KTS = {0: (7,), 1: (6, 7)}
ES_BUFS = 9

_cache = {}


def _build(flags, debug=False):
    from contextlib import ExitStack

    import concourse.bass as bass
    import concourse.mybir as mybir
    import concourse.tile as tile
    from concourse import bacc

    f32 = mybir.dt.float32
    bf16 = mybir.dt.bfloat16
    i16 = mybir.dt.int16
    AF = mybir.ActivationFunctionType
    Alu = mybir.AluOpType

    use_bq, use_bk, use_bv, use_bo, use_gam, use_bet = flags

    nc = bacc.Bacc(None, target_bir_lowering=False)

    xtq = nc.dram_tensor("xtq", [D, S], bf16, kind="ExternalInput")
    xtk = nc.dram_tensor("xtk", [D, S], bf16, kind="ExternalInput")
    xtv = nc.dram_tensor("xtv", [D, S], bf16, kind="ExternalInput")
    resid_d = nc.dram_tensor("resid", [S, D], bf16, kind="ExternalInput")
    wq = nc.dram_tensor("wq", [D, D], bf16, kind="ExternalInput")
    wk = nc.dram_tensor("wk", [D, D], bf16, kind="ExternalInput")
    wv = nc.dram_tensor("wv", [D, D], bf16, kind="ExternalInput")
    wo = nc.dram_tensor("wo", [D, D], bf16, kind="ExternalInput")
    bq = nc.dram_tensor("bq", [D], f32, kind="ExternalInput")
    bk = nc.dram_tensor("bk", [D], f32, kind="ExternalInput")
    bv = nc.dram_tensor("bv", [D], f32, kind="ExternalInput")
    bo = nc.dram_tensor("bo", [D], f32, kind="ExternalInput")
    gam = nc.dram_tensor("gam", [D], f32, kind="ExternalInput")
    bet = nc.dram_tensor("bet", [D], f32, kind="ExternalInput")
    out = nc.dram_tensor("out", [S, D], bf16, kind="ExternalOutput")

    with tile.TileContext(nc) as tc, ExitStack() as top:
        consts = top.enter_context(tc.tile_pool(name="consts", bufs=1))
        bigp = top.enter_context(tc.tile_pool(name="bigp", bufs=6))
        qkvp = top.enter_context(tc.tile_pool(name="qkvp", bufs=1))
        esp = top.enter_context(tc.tile_pool(name="esp", bufs=ES_BUFS))
        residp = top.enter_context(tc.tile_pool(name="residp", bufs=3))
        rcp = top.enter_context(tc.tile_pool(name="rcp", bufs=2))
        tmpp = top.enter_context(tc.tile_pool(name="tmpp", bufs=2))
        stp = top.enter_context(tc.tile_pool(name="stp", bufs=2))
        mvp = top.enter_context(tc.tile_pool(name="mvp", bufs=1))
        pp = top.enter_context(tc.tile_pool(name="pp", bufs=2, space="PSUM"))
        ps_s = top.enter_context(tc.tile_pool(name="ps_s", bufs=1, space="PSUM"))
        ps_c = top.enter_context(tc.tile_pool(name="ps_c", bufs=4, space="PSUM"))

        ones16 = consts.tile([P, H], f32, tag="ones16")
        nc.vector.memset(ones16[:], 1.0)
        eps_sb = consts.tile([P, 1], f32, tag="eps")
        nc.vector.memset(eps_sb[:], EPS)
        need_ones = use_bv or use_bo
        if need_ones:
            ones1 = consts.tile([1, P], bf16, tag="ones1")
            nc.vector.memset(ones1[:], 1.0)
        if use_bq:
            bq_sb = consts.tile([P, DC], f32, tag="bq")
            nc.sync.dma_start(out=bq_sb[:], in_=bq[:].rearrange("(c p) -> p c", p=P))
        if use_bk:
            bk_sb = consts.tile([P, DC], f32, tag="bk")
            nc.sync.dma_start(out=bk_sb[:], in_=bk[:].rearrange("(c p) -> p c", p=P))
        if use_bv:
            bv_f = consts.tile([1, D], f32, tag="bvf")
            nc.sync.dma_start(out=bv_f[:], in_=bv[:].rearrange("d -> 1 d"))
            bv_sb = consts.tile([1, D], bf16, tag="bv")
            nc.vector.tensor_copy(bv_sb[:], bv_f[:])
        if use_bo:
            bo_f = consts.tile([1, D], f32, tag="bof")
            nc.sync.dma_start(out=bo_f[:], in_=bo[:].rearrange("d -> 1 d"))
            bo_sb = consts.tile([1, D], bf16, tag="bo")
            nc.vector.tensor_copy(bo_sb[:], bo_f[:])
        if use_gam:
            gam_bc = consts.tile([P, D], f32, tag="gam")
            nc.sync.dma_start(
                out=gam_bc[:],
                in_=bass.AP(tensor=gam[:].tensor, offset=0, ap=[[0, P], [1, D]]),
            )
        if use_bet:
            bet_bc = consts.tile([P, D], f32, tag="bet")
            nc.sync.dma_start(
                out=bet_bc[:],
                in_=bass.AP(tensor=bet[:].tensor, offset=0, ap=[[0, P], [1, D]]),
            )

        # -------- big input tiles; allocation order = slot order -----------
        wk_sb = bigp.tile([P, DC, D], bf16, tag="big", name="wk")
        xtk_sb = bigp.tile([P, DC, S], bf16, tag="big", name="xtk")
        wv_sb = bigp.tile([P, DC, D], bf16, tag="big", name="wv")
        xtv_sb = bigp.tile([P, DC, S], bf16, tag="big", name="xtv")
        wq_sb = bigp.tile([P, DC, D], bf16, tag="big", name="wq")
        xtq_sb = bigp.tile([P, DC, S], bf16, tag="big", name="xtq")

        # DMA priority order (sync FIFO): K-side, V-side, Q-side
        for x_sb, x_d, w_sb, w_d in (
            (xtk_sb, xtk, wk_sb, wk),
            (xtv_sb, xtv, wv_sb, wv),
            (xtq_sb, xtq, wq_sb, wq),
        ):
            nc.sync.dma_start(
                out=x_sb[:], in_=x_d[:].rearrange("(c p) s -> p c s", p=P)
            )
            nc.sync.dma_start(
                out=w_sb[:], in_=w_d[:].rearrange("(c p) s -> p c s", p=P)
            )
        # wo reuses wk's slot (7th tile in a 6-buf pool) -> WAR-gated until
        # the last Kproj matmul has read wk
        wo_sb = bigp.tile([P, DC, D], bf16, tag="big", name="wo")
        nc.sync.dma_start(
            out=wo_sb[:], in_=wo[:].rearrange("(c p) s -> p c s", p=P)
        )

        qt = qkvp.tile([P, DC, S], bf16, tag="qt")
        kt = qkvp.tile([P, DC, S], bf16, tag="kt")
        vx = qkvp.tile([P, SC, H, DH + 1], bf16, tag="vx")
        ct = qkvp.tile([P, DC, S], bf16, tag="ct")
        for sc in range(SC):
            nc.vector.tensor_copy(vx[:, sc, :, DH], ones16[:])

        mv_all = mvp.tile([P, SC, 2], f32, tag="mv")
        rstd = mvp.tile([P, SC], f32, tag="rstd")

        # ------------------------- emit helpers ---------------------------
        def proj_T_half(hp, sh, w_sb, x_sb, dst, bias_sb):
            # dst[p, hp, s] = (X @ W)[s, hp*128+p]  (Q^T / K^T head-pair col)
            ssl = slice(sh * 512, (sh + 1) * 512)
            ps = pp.tile([P, 512], f32, tag="pp")
            for kc in range(DC):
                nc.tensor.matmul(
                    ps[:],
                    w_sb[:, kc, hp * P:(hp + 1) * P],
                    x_sb[:, kc, ssl],
                    start=(kc == 0),
                    stop=(kc == DC - 1),
                )
            d = dst[:, hp, ssl]
            if bias_sb is not None:
                nc.vector.tensor_scalar_add(d, in0=ps[:], scalar1=bias_sb[:, hp:hp + 1])
            else:
                nc.vector.tensor_copy(d, ps[:])

        def proj_V(sc):
            # vx[p, sc, h, d] = (Xv @ Wv)[sc*128+p, h*64+d]
            for nh in range(2):
                ps = pp.tile([P, 512], f32, tag="pp")
                for kc in range(DC):
                    nc.tensor.matmul(
                        ps[:],
                        xtv_sb[:, kc, sc * P:(sc + 1) * P],
                        wv_sb[:, kc, nh * 512:(nh + 1) * 512],
                        start=(kc == 0),
                        stop=(kc == DC - 1) and not use_bv,
                    )
                if use_bv:
                    nc.tensor.matmul(
                        ps[:], ones1[:], bv_sb[0:1, nh * 512:(nh + 1) * 512],
                        start=False, stop=True,
                    )
                dst = vx[:, sc, nh * 8:(nh + 1) * 8, 0:DH]
                nc.vector.tensor_copy(dst, ps[:].rearrange("p (h d) -> p h d", d=DH))

        es_store = {}

        def emit_scores(hp, qh, kt_i):
            qsl = slice(qh * 512, (qh + 1) * 512)
            ks = slice(kt_i * P, (kt_i + 1) * P)
            ps = ps_s.tile([P, 1024], f32, tag="ps")
            nc.tensor.matmul(
                ps[:, 0:512], kt[0:64, hp, ks], qt[0:64, hp, qsl],
                start=True, stop=True, tile_position=(0, 0),
            )
            nc.tensor.matmul(
                ps[:, 512:1024], kt[64:128, hp, ks], qt[64:128, hp, qsl],
                start=True, stop=True, tile_position=(64, 0),
            )
            es = esp.tile([P, 1024], bf16, tag="es")
            if kt_i in DVE_KTS[qh]:
                nc.vector.tensor_scalar(
                    out=es[:].bitcast(i16), in0=ps[:],
                    scalar1=EXP_A, scalar2=EXP_B,
                    op0=Alu.mult, op1=Alu.add,
                )
            else:
                nc.scalar.activation(es[:], ps[:], AF.Exp, scale=0.125)
            es_store[(hp, qh, kt_i)] = es

        ctx_ps = {}

        def ctx_pair(hp, qh, kt0):
            hA, hB = 2 * hp, 2 * hp + 1
            if kt0 == 0:
                pcA = ps_c.tile([DH + 1, 512], f32, tag="pc")
                pcB = ps_c.tile([DH + 1, 512], f32, tag="pc")
                ctx_ps[(hp, qh)] = (pcA, pcB)
            pcA, pcB = ctx_ps[(hp, qh)]
            for kt_i in (kt0, kt0 + 1):
                es = es_store.pop((hp, qh, kt_i))
                nc.tensor.matmul(
                    pcA[:], vx[:, kt_i, hA, :], es[:, 0:512],
                    start=(kt_i == 0), stop=(kt_i == SC - 1),
                )
                nc.tensor.matmul(
                    pcB[:], vx[:, kt_i, hB, :], es[:, 512:1024],
                    start=(kt_i == 0), stop=(kt_i == SC - 1),
                )

        norm_pending = []

        def normalize_begin(hp, qh):
            pcA, pcB = ctx_ps[(hp, qh)]
            rbcs = []
            for pc in (pcA, pcB):
                dn = rcp.tile([DH + 1, 512], f32, tag="dn")
                nc.vector.tensor_copy(dn[DH:DH + 1, :], pc[DH:DH + 1, :])
                dn0 = rcp.tile([1, 512], f32, tag="dn0")
                nc.sync.dma_start(out=dn0[:], in_=dn[DH:DH + 1, :])
                rbc = rcp.tile([DH, 512], f32, tag="rbc")
                nc.gpsimd.partition_broadcast(rbc[:], dn0[:])
                rbcs.append(rbc)
            norm_pending.append((hp, qh, rbcs))

        def normalize_end():
            if not norm_pending:
                return
            hp, qh, rbcs = norm_pending.pop(0)
            pcA, pcB = ctx_ps.pop((hp, qh))
            qsl = slice(qh * 512, (qh + 1) * 512)
            for h, pc, rbc in ((2 * hp, pcA, rbcs[0]), (2 * hp + 1, pcB, rbcs[1])):
                nc.vector.reciprocal_approx_fast(out=rbc[:], in_=rbc[:])
                if h % 2 == 0:
                    nc.vector.tensor_mul(ct[0:64, hp, qsl], pc[0:DH, :], rbc[:])
                else:
                    tmp = tmpp.tile([DH, 512], bf16, tag="tmp")
                    nc.vector.tensor_mul(tmp[:], pc[0:DH, :], rbc[:])
                    nc.gpsimd.dma_start(out=ct[64:128, hp, qsl], in_=tmp[:])

        op_ps = {}

        def outproj_part(sc, part):
            # part p covers mc 2p, 2p+1; PSUM groups for both feature halves
            # are completed before any pre-gelu copy touches ct (RAW safety)
            ssl = slice(sc * P, (sc + 1) * P)
            if part == 0:
                op_ps[sc] = (
                    pp.tile([P, 512], f32, tag="pp", name="poA"),
                    pp.tile([P, 512], f32, tag="pp", name="poB"),
                )
            poA, poB = op_ps[sc]
            for mc in (2 * part, 2 * part + 1):
                for po, nh in ((poA, 0), (poB, 1)):
                    nc.tensor.matmul(
                        po[:],
                        ct[:, mc, ssl],
                        wo_sb[:, mc, nh * 512:(nh + 1) * 512],
                        start=(mc == 0),
                        stop=(mc == DC - 1) and not use_bo,
                    )
            if part == 3:
                if use_bo:
                    for po, nh in ((poA, 0), (poB, 1)):
                        nc.tensor.matmul(
                            po[:], ones1[:], bo_sb[0:1, nh * 512:(nh + 1) * 512],
                            start=False, stop=True,
                        )
                for po, nh in ((poA, 0), (poB, 1)):
                    nc.vector.tensor_copy(
                        ct[:, nh * 4:(nh + 1) * 4, ssl],
                        po[:].rearrange("p (c s) -> p c s", s=P),
                    )
                del op_ps[sc]

        resid_tiles = {}

        def fetch_resid(sc):
            r = residp.tile([P, D], bf16, tag="resid")
            nc.gpsimd.dma_start(out=r[:], in_=resid_d[sc * P:(sc + 1) * P, :])
            resid_tiles[sc] = r

        # --------------------------- schedule -----------------------------
        qbias = bq_sb if use_bq else None
        with nc.named_scope("proj_k"):
            for hp in range(HP):
                proj_T_half(hp, 0, wk_sb, xtk_sb, kt, bk_sb if use_bk else None)
                proj_T_half(hp, 1, wk_sb, xtk_sb, kt, bk_sb if use_bk else None)
        with nc.named_scope("proj_v"):
            proj_T_half(0, 0, wq_sb, xtq_sb, qt, qbias)
            proj_T_half(0, 1, wq_sb, xtq_sb, qt, qbias)
            emit_scores(0, 0, 0)
            for sc in range(SC):
                proj_V(sc)
                if sc < 7:
                    emit_scores(0, 0, sc + 1)

        # 16-unit steady loop; unit u = (qh, hp); each iter also runs ctx of
        # unit u-1, normalize_begin(u-1), normalize_end (u-2), and weaves
        # Qproj halves (u<8) or outproj quarters (u>=10)
        OPCHUNK = {10: 0, 11: 1, 12: 2, 13: 3}
        with nc.named_scope("attn"):
            for u in range(1, 16):
                qh, hp = u // 8, u % 8
                pqh, php = (u - 1) // 8, (u - 1) % 8
                oc = OPCHUNK.get(u)

                if u < 8:
                    proj_T_half(hp, 0, wq_sb, xtq_sb, qt, qbias)
                elif oc is not None:
                    outproj_part(oc, 0)
                emit_scores(hp, qh, 0)
                if u < 8:
                    proj_T_half(hp, 1, wq_sb, xtq_sb, qt, qbias)
                elif oc is not None:
                    outproj_part(oc, 1)
                emit_scores(hp, qh, 1)
                ctx_pair(php, pqh, 0)
                emit_scores(hp, qh, 2)
                ctx_pair(php, pqh, 2)
                emit_scores(hp, qh, 3)
                ctx_pair(php, pqh, 4)
                emit_scores(hp, qh, 4)
                ctx_pair(php, pqh, 6)
                normalize_begin(php, pqh)
                emit_scores(hp, qh, 5)
                if oc is not None:
                    outproj_part(oc, 2)
                normalize_end()
                emit_scores(hp, qh, 6)
                if oc is not None:
                    outproj_part(oc, 3)
                emit_scores(hp, qh, 7)
                if u == 14:
                    for sc in range(3):
                        fetch_resid(sc)
            # last unit's ctx + drain the normalize pipeline
            ctx_pair(7, 1, 0)
            ctx_pair(7, 1, 2)
            ctx_pair(7, 1, 4)
            ctx_pair(7, 1, 6)
            normalize_begin(7, 1)
            normalize_end()
            normalize_end()

        with nc.named_scope("out_proj"):
            for sc in range(4, SC):
                for part in range(4):
                    outproj_part(sc, part)

            # ---- tail: deferred gelu (one ACT table switch), LN, store ----
            # y2 chunks alias the dead qt tile
            for sc in range(SC):
                ssl = slice(sc * P, (sc + 1) * P)
                y2 = qt[:, sc, :]
                nc.scalar.activation(y2, ct[:, 0:DC, ssl], AF.Gelu)
                nc.vector.tensor_add(y2, y2, resid_tiles.pop(sc)[:])
                if sc + 3 < SC:
                    fetch_resid(sc + 3)
                st = stp.tile([P, 2, 6], f32, tag="st")
                nc.vector.bn_stats(st[:, 0, :], qt[:, sc, 0:512])
                nc.vector.bn_stats(st[:, 1, :], qt[:, sc, 512:1024])
                nc.vector.bn_aggr(mv_all[:, sc, :], st[:])
            nc.scalar.activation(
                rstd[:, :], mv_all[:, :, 1], AF.Sqrt, bias=eps_sb[:]
            )
            nc.vector.reciprocal(rstd[:, :], rstd[:, :])
            for sc in range(SC):
                y2 = qt[:, sc, :]
                nc.vector.tensor_scalar(
                    out=y2, in0=y2,
                    scalar1=mv_all[:, sc, 0:1], scalar2=rstd[:, sc:sc + 1],
                    op0=Alu.subtract, op1=Alu.mult,
                )
                if use_gam:
                    nc.vector.tensor_mul(y2, y2, gam_bc[:])
                if use_bet:
                    nc.vector.tensor_add(y2, y2, bet_bc[:])
                eng = nc.sync if sc % 2 == 0 else nc.gpsimd
                eng.dma_start(out=out[sc * P:(sc + 1) * P, :], in_=y2)

    nc.finalize()
    return nc


def _get_nc(flags):
    if flags not in _cache:
        _cache[flags] = _build(flags)
    return _cache[flags]


def kernel(q, k, v, wq, bq, wk, bk, wv, bv, wo, bo, ln_gamma, ln_beta):
    import ml_dtypes
    from concourse.bass_utils import run_bass_kernel_spmd

    bf = ml_dtypes.bfloat16
    q = np.ascontiguousarray(q, dtype=np.float32)
    # host-side layout prep: X^T per batch, bf16
    qtb = np.ascontiguousarray(q.transpose(0, 2, 1)).astype(bf)
    ktb = np.ascontiguousarray(np.asarray(k, np.float32).transpose(0, 2, 1)).astype(bf)
    vtb = np.ascontiguousarray(np.asarray(v, np.float32).transpose(0, 2, 1)).astype(bf)
    residb = q.astype(bf)

    flags = (
        bool(np.any(bq)), bool(np.any(bk)), bool(np.any(bv)), bool(np.any(bo)),
        not bool(np.all(ln_gamma == 1.0)), bool(np.any(ln_beta)),
    )
    nc = _get_nc(flags)

    shared = {
        "wq": np.ascontiguousarray(wq).astype(bf),
        "wk": np.ascontiguousarray(wk).astype(bf),
        "wv": np.ascontiguousarray(wv).astype(bf),
        "wo": np.ascontiguousarray(wo).astype(bf),
        "bq": np.ascontiguousarray(bq, np.float32),
        "bk": np.ascontiguousarray(bk, np.float32),
        "bv": np.ascontiguousarray(bv, np.float32),
        "bo": np.ascontiguousarray(bo, np.float32),
        "gam": np.ascontiguousarray(ln_gamma, np.float32),
        "bet": np.ascontiguousarray(ln_beta, np.float32),
    }
    in_maps = [
        {"xtq": qtb[b], "xtk": ktb[b], "xtv": vtb[b], "resid": residb[b], **shared}
        for b in range(NCORES)
    ]
    res = run_bass_kernel_spmd(nc, in_maps, core_ids=list(range(NCORES)))
    return np.stack(
        [res.results[b]["out"].astype(np.float32) for b in range(NCORES)], axis=0
    )


# revision 6
# speedup vs baseline: 1.3973x; 1.3973x over previous
"""Trainium2 Bass kernel for a full MHA transformer block (B=8, data-parallel).

Per core (one batch element):
    qh/kh/vh = x @ W  (+zero bias), 16 heads x 64
    attn     = softmax(qh @ kh^T / 8)
    out      = LayerNorm(gelu(ctx @ Wo) + residual)

v2 design (scheduling rewrite vs the PE-transpose baseline):
  - Inputs arrive HOST-pre-transposed as X^T [d, s] bf16 -> no PE transposes.
  - DMA priority: K-side, V-side, Q-side, wo (wo reuses wk's SBUF slot, so
    its DMA is WAR-gated until Kproj finishes).
  - PE: Kproj*8 -> Qproj(0) -> Vproj*8 (scores for unit 0 woven) -> a
    16-unit steady loop (unit = (q-half, head-pair)): each iteration emits
    the unit's 8 scores tiles singly spaced between Qproj halves / ctx
    pairs / outproj quarters so the single scores-PSUM buffer round-trips
    through exp without stalling PE.
  - exp split: ScalarE activation for most kt, VectorE Schraudolph
    (i16 = A*s + B, bitcast to bf16) for the last kt(s) of each unit.
  - softmax denominator via the ones column appended to V; normalize is
    software-pipelined: [row-copy, partition-move DMA, broadcast] one unit
    ahead of [reciprocal, muls] so the cross-partition latency overlaps.
  - out-proj both-halves PSUM first, then both pre-gelu copies parked into
    the dead ct[:, :, ssl] bytes; ALL gelu deferred after the last exp
    (ACT executes in order -> exactly two activation-table switches).
  - y2 (gelu+resid, LN) aliased onto the dead qt tile; output stored bf16,
    host upcasts to f32.
"""

import numpy as np

S, D, H, DH = 1024, 1024, 16, 64
EPS = 1e-5
NCORES = 8
P = 128
SC = S // P    # seq chunks (8)
DC = D // P    # feature chunks (8)
HP = H // 2    # head pairs (8)

# Schraudolph exp as bf16 bits: i16 = A*s + B (s = raw score; the 1/8
# softmax scale is folded into A), bitcast int16 -> bf16.
EXP_A = 23.083120654223414
EXP_B = 16250.585

# which kt tiles go to the DVE instead of ACT, per q-half
DVE_KTS = {0: (7,), 1: (6, 7)}
ES_BUFS = 9

_cache = {}


def _build(flags, debug=False):
    from contextlib import ExitStack

    import concourse.bass as bass
    import concourse.mybir as mybir
    import concourse.tile as tile
    from concourse import bacc

    f32 = mybir.dt.float32
    bf16 = mybir.dt.bfloat16
    i16 = mybir.dt.int16
    AF = mybir.ActivationFunctionType
    Alu = mybir.AluOpType

    use_bq, use_bk, use_bv, use_bo, use_gam, use_bet = flags

    nc = bacc.Bacc(None, target_bir_lowering=False)

    xtq = nc.dram_tensor("xtq", [D, S], bf16, kind="ExternalInput")
    xtk = nc.dram_tensor("xtk", [D, S], bf16, kind="ExternalInput")
    xtv = nc.dram_tensor("xtv", [D, S], bf16, kind="ExternalInput")
    resid_d = nc.dram_tensor("resid", [S, D], bf16, kind="ExternalInput")
    wq = nc.dram_tensor("wq", [D, D], bf16, kind="ExternalInput")
    wk = nc.dram_tensor("wk", [D, D], bf16, kind="ExternalInput")
    wv = nc.dram_tensor("wv", [D, D], bf16, kind="ExternalInput")
    wo = nc.dram_tensor("wo", [D, D], bf16, kind="ExternalInput")
    bq = nc.dram_tensor("bq", [D], f32, kind="ExternalInput")
    bk = nc.dram_tensor("bk", [D], f32, kind="ExternalInput")
    bv = nc.dram_tensor("bv", [D], f32, kind="ExternalInput")
    bo = nc.dram_tensor("bo", [D], f32, kind="ExternalInput")
    gam = nc.dram_tensor("gam", [D], f32, kind="ExternalInput")
    bet = nc.dram_tensor("bet", [D], f32, kind="ExternalInput")
    out = nc.dram_tensor("out", [S, D], bf16, kind="ExternalOutput")

    with tile.TileContext(nc) as tc, ExitStack() as top:
        consts = top.enter_context(tc.tile_pool(name="consts", bufs=1))
        bigp = top.enter_context(tc.tile_pool(name="bigp", bufs=6))
        qkvp = top.enter_context(tc.tile_pool(name="qkvp", bufs=1))
        esp = top.enter_context(tc.tile_pool(name="esp", bufs=ES_BUFS))
        residp = top.enter_context(tc.tile_pool(name="residp", bufs=3))
        rcp = top.enter_context(tc.tile_pool(name="rcp", bufs=2))
        tmpp = top.enter_context(tc.tile_pool(name="tmpp", bufs=2))
        stp = top.enter_context(tc.tile_pool(name="stp", bufs=2))
        mvp = top.enter_context(tc.tile_pool(name="mvp", bufs=1))
        pp = top.enter_context(tc.tile_pool(name="pp", bufs=2, space="PSUM"))
        ps_s = top.enter_context(tc.tile_pool(name="ps_s", bufs=2, space="PSUM"))
        ps_c = top.enter_context(tc.tile_pool(name="ps_c", bufs=2, space="PSUM"))

        ones16 = consts.tile([P, H], f32, tag="ones16")
        nc.vector.memset(ones16[:], 1.0)
        eps_sb = consts.tile([P, 1], f32, tag="eps")
        nc.vector.memset(eps_sb[:], EPS)
        need_ones = use_bv or use_bo
        if need_ones:
            ones1 = consts.tile([1, P], bf16, tag="ones1")
            nc.vector.memset(ones1[:], 1.0)
        if use_bq:
            bq_sb = consts.tile([P, DC], f32, tag="bq")
            nc.sync.dma_start(out=bq_sb[:], in_=bq[:].rearrange("(c p) -> p c", p=P))
        if use_bk:
            bk_sb = consts.tile([P, DC], f32, tag="bk")
            nc.sync.dma_start(out=bk_sb[:], in_=bk[:].rearrange("(c p) -> p c", p=P))
        if use_bv:
            bv_f = consts.tile([1, D], f32, tag="bvf")
            nc.sync.dma_start(out=bv_f[:], in_=bv[:].rearrange("d -> 1 d"))
            bv_sb = consts.tile([1, D], bf16, tag="bv")
            nc.vector.tensor_copy(bv_sb[:], bv_f[:])
        if use_bo:
            bo_f = consts.tile([1, D], f32, tag="bof")
            nc.sync.dma_start(out=bo_f[:], in_=bo[:].rearrange("d -> 1 d"))
            bo_sb = consts.tile([1, D], bf16, tag="bo")
            nc.vector.tensor_copy(bo_sb[:], bo_f[:])
        if use_gam:
            gam_bc = consts.tile([P, D], f32, tag="gam")
            nc.sync.dma_start(
                out=gam_bc[:],
                in_=bass.AP(tensor=gam[:].tensor, offset=0, ap=[[0, P], [1, D]]),
            )
        if use_bet:
            bet_bc = consts.tile([P, D], f32, tag="bet")
            nc.sync.dma_start(
                out=bet_bc[:],
                in_=bass.AP(tensor=bet[:].tensor, offset=0, ap=[[0, P], [1, D]]),
            )

        # -------- big input tiles; allocation order = slot order -----------
        wk_sb = bigp.tile([P, DC, D], bf16, tag="big", name="wk")
        xtk_sb = bigp.tile([P, DC, S], bf16, tag="big", name="xtk")
        wv_sb = bigp.tile([P, DC, D], bf16, tag="big", name="wv")
        xtv_sb = bigp.tile([P, DC, S], bf16, tag="big", name="xtv")
        wq_sb = bigp.tile([P, DC, D], bf16, tag="big", name="wq")
        xtq_sb = bigp.tile([P, DC, S], bf16, tag="big", name="xtq")

        # DMA priority order (sync FIFO): K-side, V-side, Q-side
        for x_sb, x_d, w_sb, w_d in (
            (xtk_sb, xtk, wk_sb, wk),
            (xtv_sb, xtv, wv_sb, wv),
            (xtq_sb, xtq, wq_sb, wq),
        ):
            nc.sync.dma_start(
                out=x_sb[:], in_=x_d[:].rearrange("(c p) s -> p c s", p=P)
            )
            nc.sync.dma_start(
                out=w_sb[:], in_=w_d[:].rearrange("(c p) s -> p c s", p=P)
            )
        # wo reuses wk's slot (7th tile in a 6-buf pool) -> WAR-gated until
        # the last Kproj matmul has read wk
        wo_sb = bigp.tile([P, DC, D], bf16, tag="big", name="wo")
        nc.sync.dma_start(
            out=wo_sb[:], in_=wo[:].rearrange("(c p) s -> p c s", p=P)
        )

        qt = qkvp.tile([P, DC, S], bf16, tag="qt")
        kt = qkvp.tile([P, DC, S], bf16, tag="kt")
        vx = qkvp.tile([P, SC, H, DH + 1], bf16, tag="vx")
        ct = qkvp.tile([P, DC, S], bf16, tag="ct")
        for sc in range(SC):
            nc.vector.tensor_copy(vx[:, sc, :, DH], ones16[:])

        mv_all = mvp.tile([P, SC, 2], f32, tag="mv")
        rstd = mvp.tile([P, SC], f32, tag="rstd")

        # ------------------------- emit helpers ---------------------------
        def proj_T_half(hp, sh, w_sb, x_sb, dst, bias_sb):
            # dst[p, hp, s] = (X @ W)[s, hp*128+p]  (Q^T / K^T head-pair col)
            ssl = slice(sh * 512, (sh + 1) * 512)
            ps = pp.tile([P, 512], f32, tag="pp")
            for kc in range(DC):
                nc.tensor.matmul(
                    ps[:],
                    w_sb[:, kc, hp * P:(hp + 1) * P],
                    x_sb[:, kc, ssl],
                    start=(kc == 0),
                    stop=(kc == DC - 1),
                )
            d = dst[:, hp, ssl]
            if bias_sb is not None:
                nc.vector.tensor_scalar_add(d, in0=ps[:], scalar1=bias_sb[:, hp:hp + 1])
            else:
                nc.vector.tensor_copy(d, ps[:])

        def proj_V(sc):
            # vx[p, sc, h, d] = (Xv @ Wv)[sc*128+p, h*64+d]
            for nh in range(2):
                ps = pp.tile([P, 512], f32, tag="pp")
                for kc in range(DC):
                    nc.tensor.matmul(
                        ps[:],
                        xtv_sb[:, kc, sc * P:(sc + 1) * P],
                        wv_sb[:, kc, nh * 512:(nh + 1) * 512],
                        start=(kc == 0),
                        stop=(kc == DC - 1) and not use_bv,
                    )
                if use_bv:
                    nc.tensor.matmul(
                        ps[:], ones1[:], bv_sb[0:1, nh * 512:(nh + 1) * 512],
                        start=False, stop=True,
                    )
                dst = vx[:, sc, nh * 8:(nh + 1) * 8, 0:DH]
                nc.vector.tensor_copy(dst, ps[:].rearrange("p (h d) -> p h d", d=DH))

        es_store = {}

        def emit_scores(hp, qh, kt_i):
            qsl = slice(qh * 512, (qh + 1) * 512)
            ks = slice(kt_i * P, (kt_i + 1) * P)
            ps = ps_s.tile([P, 1024], f32, tag="ps")
            nc.tensor.matmul(
                ps[:, 0:512], kt[0:64, hp, ks], qt[0:64, hp, qsl],
                start=True, stop=True, tile_position=(0, 0),
            )
            nc.tensor.matmul(
                ps[:, 512:1024], kt[64:128, hp, ks], qt[64:128, hp, qsl],
                start=True, stop=True, tile_position=(64, 0),
            )
            es = esp.tile([P, 1024], bf16, tag="es")
            if kt_i in DVE_KTS[qh]:
                nc.vector.tensor_scalar(
                    out=es[:].bitcast(i16), in0=ps[:],
                    scalar1=EXP_A, scalar2=EXP_B,
                    op0=Alu.mult, op1=Alu.add,
                )
            else:
                nc.scalar.activation(es[:], ps[:], AF.Exp, scale=0.125)
            es_store[(hp, qh, kt_i)] = es

        ctx_ps = {}

        def ctx_pair(hp, qh, kt0):
            hA, hB = 2 * hp, 2 * hp + 1
            if kt0 == 0:
                pcA = ps_c.tile([DH + 1, 512], f32, tag="pc")
                pcB = ps_c.tile([DH + 1, 512], f32, tag="pc")
                ctx_ps[(hp, qh)] = (pcA, pcB)
            pcA, pcB = ctx_ps[(hp, qh)]
            for kt_i in (kt0, kt0 + 1):
                es = es_store.pop((hp, qh, kt_i))
                nc.tensor.matmul(
                    pcA[:], vx[:, kt_i, hA, :], es[:, 0:512],
                    start=(kt_i == 0), stop=(kt_i == SC - 1),
                )
                nc.tensor.matmul(
                    pcB[:], vx[:, kt_i, hB, :], es[:, 512:1024],
                    start=(kt_i == 0), stop=(kt_i == SC - 1),
                )

        norm_pending = []

        def normalize_begin(hp, qh):
            pcA, pcB = ctx_ps[(hp, qh)]
            rbcs = []
            for pc in (pcA, pcB):
                dn = rcp.tile([DH + 1, 512], f32, tag="dn")
                nc.vector.tensor_copy(dn[DH:DH + 1, :], pc[DH:DH + 1, :])
                dn0 = rcp.tile([1, 512], f32, tag="dn0")
                nc.sync.dma_start(out=dn0[:], in_=dn[DH:DH + 1, :])
                rbc = rcp.tile([DH, 512], f32, tag="rbc")
                nc.gpsimd.partition_broadcast(rbc[:], dn0[:])
                rbcs.append(rbc)
            norm_pending.append((hp, qh, rbcs))

        def normalize_end():
            if not norm_pending:
                return
            hp, qh, rbcs = norm_pending.pop(0)
            pcA, pcB = ctx_ps.pop((hp, qh))
            qsl = slice(qh * 512, (qh + 1) * 512)
            for h, pc, rbc in ((2 * hp, pcA, rbcs[0]), (2 * hp + 1, pcB, rbcs[1])):
                nc.vector.reciprocal_approx_fast(out=rbc[:], in_=rbc[:])
                if h % 2 == 0:
                    nc.vector.tensor_mul(ct[0:64, hp, qsl], pc[0:DH, :], rbc[:])
                else:
                    tmp = tmpp.tile([DH, 512], bf16, tag="tmp")
                    nc.vector.tensor_mul(tmp[:], pc[0:DH, :], rbc[:])
                    nc.gpsimd.dma_start(out=ct[64:128, hp, qsl], in_=tmp[:])

        op_ps = {}

        def outproj_part(sc, part):
            # part p covers mc 2p, 2p+1; PSUM groups for both feature halves
            # are completed before any pre-gelu copy touches ct (RAW safety)
            ssl = slice(sc * P, (sc + 1) * P)
            if part == 0:
                op_ps[sc] = (
                    pp.tile([P, 512], f32, tag="pp", name="poA"),
                    pp.tile([P, 512], f32, tag="pp", name="poB"),
                )
            poA, poB = op_ps[sc]
            for mc in (2 * part, 2 * part + 1):
                for po, nh in ((poA, 0), (poB, 1)):
                    nc.tensor.matmul(
                        po[:],
                        ct[:, mc, ssl],
                        wo_sb[:, mc, nh * 512:(nh + 1) * 512],
                        start=(mc == 0),
                        stop=(mc == DC - 1) and not use_bo,
                    )
            if part == 3:
                if use_bo:
                    for po, nh in ((poA, 0), (poB, 1)):
                        nc.tensor.matmul(
                            po[:], ones1[:], bo_sb[0:1, nh * 512:(nh + 1) * 512],
                            start=False, stop=True,
                        )
                for po, nh in ((poA, 0), (poB, 1)):
                    nc.vector.tensor_copy(
                        ct[:, nh * 4:(nh + 1) * 4, ssl],
                        po[:].rearrange("p (c s) -> p c s", s=P),
                    )
                del op_ps[sc]

        resid_tiles = {}

        def fetch_resid(sc):
            r = residp.tile([P, D], bf16, tag="resid")
            nc.gpsimd.dma_start(out=r[:], in_=resid_d[sc * P:(sc + 1) * P, :])
            resid_tiles[sc] = r

        # --------------------------- schedule -----------------------------
        qbias = bq_sb if use_bq else None
        with nc.named_scope("proj_k"):
            for hp in range(HP):
                proj_T_half(hp, 0, wk_sb, xtk_sb, kt, bk_sb if use_bk else None)
                proj_T_half(hp, 1, wk_sb, xtk_sb, kt, bk_sb if use_bk else None)
        with nc.named_scope("proj_v"):
            proj_T_half(0, 0, wq_sb, xtq_sb, qt, qbias)
            proj_T_half(0, 1, wq_sb, xtq_sb, qt, qbias)
            emit_scores(0, 0, 0)
            for sc in range(SC):
                proj_V(sc)
                if sc < 7:
                    emit_scores(0, 0, sc + 1)

        # 16-unit steady loop; unit u = (qh, hp); each iter also runs ctx of
        # unit u-1, normalize_begin(u-1), normalize_end (u-2), and weaves
        # Qproj halves (u<8) or outproj quarters (u>=10)
        OPCHUNK = {10: 0, 11: 1, 12: 2, 13: 3}
        with nc.named_scope("attn"):
            for u in range(1, 16):
                qh, hp = u // 8, u % 8
                pqh, php = (u - 1) // 8, (u - 1) % 8
                oc = OPCHUNK.get(u)

                # free the two ctx-PSUM tiles of unit u-2 before unit u-1
                # claims them (normalize latency already covered by iter u-1)
                normalize_end()
                if u < 8:
                    proj_T_half(hp, 0, wq_sb, xtq_sb, qt, qbias)
                elif oc is not None:
                    outproj_part(oc, 0)
                emit_scores(hp, qh, 0)
                if u < 8:
                    proj_T_half(hp, 1, wq_sb, xtq_sb, qt, qbias)
                elif oc is not None:
                    outproj_part(oc, 1)
                emit_scores(hp, qh, 1)
                ctx_pair(php, pqh, 0)
                emit_scores(hp, qh, 2)
                ctx_pair(php, pqh, 2)
                emit_scores(hp, qh, 3)
                ctx_pair(php, pqh, 4)
                emit_scores(hp, qh, 4)
                if oc is not None:
                    outproj_part(oc, 2)
                ctx_pair(php, pqh, 6)
                normalize_begin(php, pqh)
                emit_scores(hp, qh, 5)
                emit_scores(hp, qh, 6)
                if oc is not None:
                    outproj_part(oc, 3)
                emit_scores(hp, qh, 7)
                if u == 14:
                    for sc in range(3):
                        fetch_resid(sc)
            # last unit's ctx + drain the normalize pipeline
            ctx_pair(7, 1, 0)
            ctx_pair(7, 1, 2)
            ctx_pair(7, 1, 4)
            ctx_pair(7, 1, 6)
            normalize_begin(7, 1)
            normalize_end()
            normalize_end()

        with nc.named_scope("out_proj"):
            for sc in range(4, SC):
                for part in range(4):
                    outproj_part(sc, part)

            # ---- tail: deferred gelu (one ACT table switch), LN, store ----
            # y2 chunks alias the dead qt tile
            for sc in range(SC):
                ssl = slice(sc * P, (sc + 1) * P)
                y2 = qt[:, sc, :]
                nc.scalar.activation(y2, ct[:, 0:DC, ssl], AF.Gelu)
                nc.vector.tensor_add(y2, y2, resid_tiles.pop(sc)[:])
                if sc + 3 < SC:
                    fetch_resid(sc + 3)
                st = stp.tile([P, 2, 6], f32, tag="st")
                nc.vector.bn_stats(st[:, 0, :], qt[:, sc, 0:512])
                nc.vector.bn_stats(st[:, 1, :], qt[:, sc, 512:1024])
                nc.vector.bn_aggr(mv_all[:, sc, :], st[:])
            nc.scalar.activation(
                rstd[:, :], mv_all[:, :, 1], AF.Sqrt, bias=eps_sb[:]
            )
            nc.vector.reciprocal(rstd[:, :], rstd[:, :])
            for sc in range(SC):
                y2 = qt[:, sc, :]
                nc.vector.tensor_scalar(
                    out=y2, in0=y2,
                    scalar1=mv_all[:, sc, 0:1], scalar2=rstd[:, sc:sc + 1],
                    op0=Alu.subtract, op1=Alu.mult,
                )
                if use_gam:
                    nc.vector.tensor_mul(y2, y2, gam_bc[:])
                if use_bet:
                    nc.vector.tensor_add(y2, y2, bet_bc[:])
                eng = nc.sync if sc % 2 == 0 else nc.gpsimd
                eng.dma_start(out=out[sc * P:(sc + 1) * P, :], in_=y2)

    nc.finalize()
    return nc


def _get_nc(flags):
    if flags not in _cache:
        _cache[flags] = _build(flags)
    return _cache[flags]


def kernel(q, k, v, wq, bq, wk, bk, wv, bv, wo, bo, ln_gamma, ln_beta):
    import ml_dtypes
    from concourse.bass_utils import run_bass_kernel_spmd

    bf = ml_dtypes.bfloat16
    q = np.ascontiguousarray(q, dtype=np.float32)
    # host-side layout prep: X^T per batch, bf16
    qtb = np.ascontiguousarray(q.transpose(0, 2, 1)).astype(bf)
    ktb = np.ascontiguousarray(np.asarray(k, np.float32).transpose(0, 2, 1)).astype(bf)
    vtb = np.ascontiguousarray(np.asarray(v, np.float32).transpose(0, 2, 1)).astype(bf)
    residb = q.astype(bf)

    flags = (
        bool(np.any(bq)), bool(np.any(bk)), bool(np.any(bv)), bool(np.any(bo)),
        not bool(np.all(ln_gamma == 1.0)), bool(np.any(ln_beta)),
    )
    nc = _get_nc(flags)

    shared = {
        "wq": np.ascontiguousarray(wq).astype(bf),
        "wk": np.ascontiguousarray(wk).astype(bf),
        "wv": np.ascontiguousarray(wv).astype(bf),
        "wo": np.ascontiguousarray(wo).astype(bf),
        "bq": np.ascontiguousarray(bq, np.float32),
        "bk": np.ascontiguousarray(bk, np.float32),
        "bv": np.ascontiguousarray(bv, np.float32),
        "bo": np.ascontiguousarray(bo, np.float32),
        "gam": np.ascontiguousarray(ln_gamma, np.float32),
        "bet": np.ascontiguousarray(ln_beta, np.float32),
    }
    in_maps = [
        {"xtq": qtb[b], "xtk": ktb[b], "xtv": vtb[b], "resid": residb[b], **shared}
        for b in range(NCORES)
    ]
    res = run_bass_kernel_spmd(nc, in_maps, core_ids=list(range(NCORES)))
    return np.stack(
        [res.results[b]["out"].astype(np.float32) for b in range(NCORES)], axis=0
    )
